# revision 27
# baseline (speedup 1.0000x reference)
"""GCNConv Trainium2 kernel (8 NeuronCores, Bass/Tile).

out = relu( D^{-1/2} (A + I) D^{-1/2} (x W^T + b) )

Distribution: destination nodes (output rows) are sharded across 8 cores.
Edges are partitioned by destination row so the segment-sum is core-local.
x is replicated to every core's HBM; each core gathers the source rows it
needs via the SWDGE dma_gather instruction. The small weight/bias are
replicated.

Device algorithm per core (dest rows R_m, |R_m| = N/8):
  reorder:  agg[n] = sum_{e: dst=n} norm[e] * x[src[e]]      (gather + one-hot matmul)
            out[n] = relu( agg[n] @ W^T + P1[n] * b )        (P1[n] = sum norm over row n)
  where norm/P1 (pure degree-normalization scalars) are computed on host as
  part of the edge partitioning pass; self-loops are folded in as edges.

Segment-sum on device: each core's destinations are packed into groups of
<=128 (greedy assignment balancing per-bank edge counts); a group's edges
are processed in chunks of 128 (one edge per SBUF partition). For each chunk
a selection matrix S[e, d] = norm[e] * (slot_local[e] == d) is built on the
vector engine from a constant iota row, then PE computes aggT += G^T S with
PSUM accumulation over the group's chunks. Groups are processed in pairs
sharing a [128, 256] PSUM tile so the matmuls run with a 256-wide moving
operand, which lets the float32r (TF32-style) path stream one row per cycle
(4x over plain fp32). A second PE matmul applies W plus the bias
outer-product, ScalarE applies relu, and per-pair DMAs store the core's
transposed output slab, which the host un-permutes.

dma_gather uses int16 indices, so the gather source x is addressed in banks
of 32768 rows; each (group, bank) segment is padded to a multiple of 128
edges, and the per-bank chunk count is uniform across groups and cores so
all cores run one SPMD program. Each core's x copy is rolled by its row
offset so self-loop columns always land in bank 0, keeping per-core bank
loads uniform (chunk counts are data-derived maxima).
"""

import math

import numpy as np

_N_CORES = 8
_P = 128  # partitions / feature dim / dest-group width
_BANK = 32768  # int16-addressable rows per gather bank
_GB = 4  # dest groups per gather batch
_NS = 8  # selection-tile ring depth per parity
_GBUFS = 2  # gather pool buffers
_PS1BUFS = 2  # phase-1 psum buffers
_SPLIT_OUT = True  # store output per pair instead of one slab

_program_cache: dict = {}
_ABLATE = "full"  # dev knob: "full" | "gather" | "compute"


# ---------------------------------------------------------------- host prep

def _host_prep(x, W, b, edge_weight, edge_index, n_cores):
    N, D = x.shape
    assert D == _P
    assert N % n_cores == 0
    nd = N // n_cores  # dest rows per core
    G = math.ceil(nd / _P)  # dest groups per core
    NB = math.ceil(N / _BANK)  # gather banks

    ei = np.asarray(edge_index)
    row = np.concatenate([np.arange(N, dtype=np.int64), ei[0].astype(np.int64)])
    col = np.concatenate([np.arange(N, dtype=np.int64), ei[1].astype(np.int64)])
    w = np.concatenate(
        [np.ones(N, np.float64), np.asarray(edge_weight, np.float64)]
    )

    deg = np.bincount(row, weights=w, minlength=N)
    d_inv = np.where(deg > 0, 1.0 / np.sqrt(np.maximum(deg, 1e-300)), 0.0)
    norm = d_inv[row] * w * d_inv[col]
    p1 = np.bincount(row, weights=norm, minlength=N).astype(np.float32)

    # Each core gathers from its own rolled copy of x (core m holds
    # x[(r + m*nd) % N] at row r), so its column indices are shifted by
    # -m*nd. This puts every core's self-loop columns in bank 0 and keeps
    # per-core bank loads uniform, minimizing the uniform chunk counts.
    core_eg = row // nd
    col = (col - core_eg * nd) % N

    # --- balanced dest->group assignment (per core) ---
    # Greedily pack each core's dests into G groups of <=128, balancing the
    # per-bank edge counts (the last bank is the tight constraint) so the
    # uniform per-bank chunk counts carry minimal padding.
    import heapq

    ebank = col // _BANK
    d_b = np.zeros((NB, N), np.int64)
    for bb in range(NB):
        d_b[bb] = np.bincount(row[ebank == bb], minlength=N)
    d_last = d_b[-1].reshape(n_cores, nd)
    d_rest = d_b[:-1].sum(axis=0).reshape(n_cores, nd) if NB > 1 else np.zeros(
        (n_cores, nd), np.int64
    )

    grp_of = np.zeros((n_cores, nd), np.int64)
    slot_of = np.zeros((n_cores, nd), np.int64)
    for m in range(n_cores):
        dl_last = d_last[m]
        dl_rest = d_rest[m]
        cnt = np.zeros(G, np.int64)
        bl = np.zeros(G, np.int64)  # last-bank load
        br = np.zeros(G, np.int64)  # other-banks load
        # phase 1: dests with last-bank edges, heaviest first, balance (bl, br)
        # phase 2: remaining dests, balance br
        p1_ids = np.where(dl_last > 0)[0]
        p1_ids = p1_ids[np.lexsort((-dl_rest[p1_ids], -dl_last[p1_ids]))]
        p2_ids = np.where(dl_last == 0)[0]
        p2_ids = p2_ids[np.argsort(-dl_rest[p2_ids], kind="stable")]
        heap = [(0, 0, g) for g in range(G)]
        for dl in p1_ids:
            while True:
                b1v, b0v, g = heapq.heappop(heap)
                if b1v == bl[g] and b0v == br[g] and cnt[g] < _P:
                    break
            grp_of[m, dl] = g
            slot_of[m, dl] = cnt[g]
            cnt[g] += 1
            bl[g] += dl_last[dl]
            br[g] += dl_rest[dl]
            if cnt[g] < _P:
                heapq.heappush(heap, (bl[g], br[g], g))
        heap = [(br[g], g) for g in range(G) if cnt[g] < _P]
        heapq.heapify(heap)
        for dl in p2_ids:
            while True:
                b0v, g = heapq.heappop(heap)
                if b0v == br[g] and cnt[g] < _P:
                    break
            grp_of[m, dl] = g
            slot_of[m, dl] = cnt[g]
            cnt[g] += 1
            br[g] += dl_rest[dl]
            if cnt[g] < _P:
                heapq.heappush(heap, (br[g], g))
    # pos in padded [G*128] output space
    pos_of = grp_of * _P + slot_of  # [M, nd]

    bank = col // _BANK
    core_e = row // nd
    loc_e = row - core_e * nd
    grp_e = grp_of[core_e, loc_e]
    slot_e = slot_of[core_e, loc_e].astype(np.float32)

    order = np.lexsort((bank, core_e * G + grp_e))
    cs = col[order]
    bs = bank[order]
    ns = norm[order].astype(np.float32)
    core_s = core_e[order]
    grp_s = grp_e[order]
    slot_s = slot_e[order]

    gid2 = (core_s * G + grp_s) * NB + bs  # sorted ascending
    counts = np.bincount(gid2, minlength=n_cores * G * NB).reshape(-1, NB)
    K = np.maximum(1, np.ceil(counts.max(axis=0) / _P).astype(np.int64))
    Ktot = int(K.sum())
    C = G * Ktot

    # chunk index: batch-major, bank-major within batch
    # batch t covers groups [t*GB, min((t+1)*GB, G)); base chunk = g0*Ktot
    # bank0 chunks of batch at base + (g-g0)*K0 + k ; bank1 after all bank0.
    g0_of = (grp_s // _GB) * _GB
    gin = grp_s - g0_of
    gsz = np.minimum(G - g0_of, _GB)  # groups in this batch
    Kpre = np.zeros(NB + 1, np.int64)
    Kpre[1:] = np.cumsum(K)

    starts = np.zeros(n_cores * G * NB, np.int64)
    starts[1:] = np.cumsum(counts.reshape(-1))[:-1]
    s = np.arange(len(cs), dtype=np.int64) - starts[gid2]
    k = s // _P
    p = s - k * _P
    c = g0_of * Ktot + gsz * Kpre[bs] + gin * K[bs] + k

    dest_arr = np.zeros((n_cores, _P, C), np.float32)
    norm_arr = np.zeros((n_cores, _P, C), np.float32)
    flat = (core_s * _P + p) * C + c
    # groups are processed in pairs sharing a [128, 256] selection matrix;
    # odd group of each pair targets columns 128..255
    dest_arr.reshape(-1)[flat] = slot_s + _P * (grp_s % 2)
    norm_arr.reshape(-1)[flat] = ns

    # int16 gather indices: flat slot j = c*128 + p -> idx16[j%16, j//16]
    idx16 = np.zeros((n_cores, 16, C * 8), np.int16)
    sflat = c * _P + p
    iflat = (core_s * 16 + sflat % 16) * (C * 8) + sflat // 16
    idx16.reshape(-1)[iflat] = (cs - bs * _BANK).astype(np.int16)
    idx_tile = np.tile(idx16, (1, 8, 1))  # replicate down 128 partitions

    NP = math.ceil(G / 2)  # group pairs
    p1_arr = np.zeros((n_cores, 1, NP * 2 * _P), np.float32)
    mrows2 = np.repeat(np.arange(n_cores), nd)
    p1_arr.reshape(n_cores, -1)[mrows2, pos_of.reshape(-1)] = p1.reshape(-1)

    iota = np.tile(np.arange(2 * _P, dtype=np.float32), (_P, 1))
    wT = np.ascontiguousarray(np.asarray(W, np.float32).T)
    bias = np.asarray(b, np.float32).reshape(1, _P)
    x_f32 = np.ascontiguousarray(np.asarray(x, np.float32))

    cfg = (N, nd, G, tuple(int(v) for v in K), n_cores)
    in_maps = []
    for m in range(n_cores):
        x_m = np.roll(x_f32, -m * nd, axis=0) if m else x_f32
        in_maps.append(
            {
                "x": x_m,
                "idx": idx_tile[m],
                "dest": dest_arr[m],
                "enorm": norm_arr[m],
                "p1": p1_arr[m],
                "wT": wT,
                "bias": bias,
                "iota": iota,
            }
        )
    return cfg, in_maps, pos_of


# ---------------------------------------------------------------- device program

def _build_program(cfg):
    from concourse import bacc, mybir, tile

    N, nd, G, K, n_cores = cfg
    NB = len(K)
    Ktot = sum(K)
    C = G * Ktot
    NP = math.ceil(G / 2)  # group pairs ([128, 256] psum per pair)
    W2 = 2 * _P
    f32 = mybir.dt.float32
    f32r = mybir.dt.float32r
    i16 = mybir.dt.int16

    nc = bacc.Bacc(
        "TRN2",
        target_bir_lowering=False,
        debug=False,
        enable_asserts=False,
        num_devices=n_cores,
    )
    x_d = nc.dram_tensor("x", [N, _P], f32r, kind="ExternalInput").ap()
    idx_d = nc.dram_tensor("idx", [_P, C * 8], i16, kind="ExternalInput").ap()
    dest_d = nc.dram_tensor("dest", [_P, C], f32, kind="ExternalInput").ap()
    norm_d = nc.dram_tensor("enorm", [_P, C], f32, kind="ExternalInput").ap()
    p1_d = nc.dram_tensor("p1", [1, NP * W2], f32r, kind="ExternalInput").ap()
    wT_d = nc.dram_tensor("wT", [_P, _P], f32r, kind="ExternalInput").ap()
    b_d = nc.dram_tensor("bias", [1, _P], f32r, kind="ExternalInput").ap()
    iota_d = nc.dram_tensor("iota", [_P, W2], f32, kind="ExternalInput").ap()
    out_d = nc.dram_tensor("outT", [_P, NP * W2], f32, kind="ExternalOutput").ap()

    NTB = math.ceil(G / _GB)  # gather batches

    with tile.TileContext(nc) as tc:
        with (
            tc.tile_pool(name="const", bufs=1) as cpool,
            tc.tile_pool(name="gather", bufs=_GBUFS) as gpool,
            tc.tile_pool(name="agg", bufs=2) as apool,
            tc.tile_pool(name="ps1", bufs=_PS1BUFS, space="PSUM") as ps1pool,
            tc.tile_pool(name="ps2", bufs=2, space="PSUM") as ps2pool,
        ):
            iota_t = cpool.tile([_P, W2], f32)
            nc.sync.dma_start(out=iota_t[:], in_=iota_d)
            wT_t = cpool.tile([_P, _P], f32r)
            nc.sync.dma_start(out=wT_t[:], in_=wT_d)
            b_t = cpool.tile([1, _P], f32r)
            nc.sync.dma_start(out=b_t[:], in_=b_d)
            p1_t = cpool.tile([1, NP * W2], f32r)
            nc.sync.dma_start(out=p1_t[:], in_=p1_d)
            idx_t = cpool.tile([_P, C * 8], i16)
            dest_t = cpool.tile([_P, C], f32)
            norm_t = cpool.tile([_P, C], f32)
            # load the first batch's indices separately so gathers start early
            cb1 = min(G, _GB) * Ktot
            nc.sync.dma_start(out=idx_t[:, : cb1 * 8], in_=idx_d[:, : cb1 * 8])
            if cb1 < C:
                nc.sync.dma_start(
                    out=idx_t[:, cb1 * 8 :], in_=idx_d[:, cb1 * 8 :]
                )
            nc.sync.dma_start(out=dest_t[:], in_=dest_d)
            nc.sync.dma_start(out=norm_t[:], in_=norm_d)
            out_t = cpool.tile([_P, NP * W2], f32)

            # persistent selection tiles: even-group tiles keep cols 128..255
            # zero forever, odd-group tiles keep cols 0..127 zero
            NS = _NS
            s_tiles = [[], []]
            for half in range(2):
                for i in range(NS):
                    st = cpool.tile([_P, W2], f32r, tag=f"s{half}_{i}")
                    nc.vector.memset(st[:].bitcast(f32), 0.0)
                    s_tiles[half].append(st)
            s_rr = [0, 0]

            for t in range(NTB):
                g0 = t * _GB
                g1 = min(g0 + _GB, G)
                gsz = g1 - g0
                base = g0 * Ktot
                gts = []
                for bkid in range(NB):
                    nch = gsz * K[bkid]
                    c0 = base + gsz * sum(K[:bkid])
                    gt = gpool.tile([_P, _GB * K[bkid] * _P], f32r, tag=f"g{bkid}")
                    lo = bkid * _BANK
                    hi = min(N, lo + _BANK)
                    nc.gpsimd.dma_gather(
                        out_ap=gt[:, : nch * _P].rearrange(
                            "p (c e) -> p c e", e=_P
                        ),
                        in_ap=x_d[lo:hi, :],
                        idxs_ap=idx_t[:, c0 * 8 : (c0 + nch) * 8],
                        num_idxs=nch * _P,
                        num_idxs_reg=nch * _P,
                        elem_size=_P,
                        single_packet=False,
                    ) if _ABLATE != "compute" else None
                    gts.append(gt)
                for pg0 in range(g0, g1, 2):
                    pr = pg0 // 2
                    pgrp = [g for g in (pg0, pg0 + 1) if g < g1]
                    ps1 = ps1pool.tile([_P, W2], f32, tag="ps1")
                    nmm = sum(K) * len(pgrp)
                    imm = 0
                    if _ABLATE == "gather":
                        continue
                    for g in pgrp:
                        half = g % 2
                        for bkid in range(NB):
                            for k in range(K[bkid]):
                                c = (
                                    base
                                    + gsz * sum(K[:bkid])
                                    + (g - g0) * K[bkid]
                                    + k
                                )
                                cl = (g - g0) * K[bkid] + k
                                S = s_tiles[half][s_rr[half]]
                                s_rr[half] = (s_rr[half] + 1) % NS
                                nc.vector.tensor_scalar(
                                    out=S[:, half * _P : (half + 1) * _P],
                                    in0=iota_t[:, half * _P : (half + 1) * _P],
                                    scalar1=dest_t[:, c : c + 1],
                                    scalar2=norm_t[:, c : c + 1],
                                    op0=mybir.AluOpType.is_equal,
                                    op1=mybir.AluOpType.mult,
                                )
                                nc.tensor.matmul(
                                    out=ps1[:],
                                    lhsT=gts[bkid][
                                        :, cl * _P : (cl + 1) * _P
                                    ],
                                    rhs=S[:],
                                    start=(imm == 0),
                                    stop=(imm == nmm - 1),
                                )
                                imm += 1
                    aggT = apool.tile([_P, W2], f32r, tag="a")
                    nc.scalar.copy(out=aggT[:], in_=ps1[:])
                    ps2 = ps2pool.tile([_P, W2], f32, tag="ps2")
                    nc.tensor.matmul(
                        out=ps2[:],
                        lhsT=wT_t[:],
                        rhs=aggT[:],
                        start=True,
                        stop=False,
                    )
                    nc.tensor.matmul(
                        out=ps2[:],
                        lhsT=b_t[:],
                        rhs=p1_t[:, pr * W2 : (pr + 1) * W2],
                        start=False,
                        stop=True,
                    )
                    nc.scalar.activation(
                        out=out_t[:, pr * W2 : (pr + 1) * W2],
                        in_=ps2[:],
                        func=mybir.ActivationFunctionType.Relu,
                    )
                    if _SPLIT_OUT:
                        nc.sync.dma_start(
                            out=out_d[:, pr * W2 : (pr + 1) * W2],
                            in_=out_t[:, pr * W2 : (pr + 1) * W2],
                        )
            if _ABLATE == "gather":
                nc.vector.memset(out_t[:, :2], 0.0)
                nc.sync.dma_start(out=out_d[:, :2], in_=out_t[:, :2])
            elif not _SPLIT_OUT:
                nc.sync.dma_start(out=out_d, in_=out_t[:])

    nc.compile()
    return nc


def _get_program(cfg):
    if cfg not in _program_cache:
        _program_cache[cfg] = _build_program(cfg)
    return _program_cache[cfg]


# ---------------------------------------------------------------- entry points

def run(inputs: dict, trace: bool = False, n_cores: int = _N_CORES):
    """Run the kernel; returns (full_output, BassKernelResults)."""
    from concourse import bass_utils

    cfg, in_maps, pos_of = _host_prep(
        inputs["x"],
        inputs["W"],
        inputs["b"],
        inputs["edge_weight"],
        inputs["edge_index"],
        n_cores,
    )
    nc = _get_program(cfg)
    res = bass_utils.run_bass_kernel_spmd(
        nc, in_maps, core_ids=list(range(n_cores)), trace=trace
    )
    N, nd = cfg[0], cfg[1]
    out = np.empty((N, _P), np.float32)
    for m in range(n_cores):
        slab = res.results[m]["outT"].T  # [NP*256, 128]
        out[m * nd : (m + 1) * nd, :] = slab[pos_of[m]]
    return out, res


def kernel(**inputs) -> np.ndarray:
    out, _ = run(inputs, trace=False)
    return out
